# revision 14
# baseline (speedup 1.0000x reference)
"""Trainium2 Bass kernel for nn_NewAttention (Gaussian-window "attention").

Math (from the reference):
    v   = values @ Wi^T                      # [B,S,E] value projection
    y2  = per-head 5-tap Gaussian conv along S of v, head h's window center
          displaced by HEAD_SHIFTS[h]
    out = y2 @ Wo^T

Key restructuring for the hardware:
  * keys/queries are unused -> never transferred.
  * Data parallel over batch: core b handles values[b].
  * The per-head displacement is a pure translation of the (uniform,
    symmetric) 5-tap filter, and the filter conv along S commutes with the
    position-wise projection.  So:
        y2[:, head h cols] = (Wi_h @ conv0(values^T))[:, s + shift_h]
    We permute channels so each 128-row tile of the projected space has a
    uniform shift (heads grouped [0,3 | 4,7 | 1,5 | 2,6]), making the shift
    a free AP column offset on the matmul rhs.
  * conv0 runs on DVE/GPSIMD as a 4-op fused chain per tile; the center tap
    scale c0 is folded into Wi on the host.
  * values^T (padded) is produced host-side during input sharding, so both
    matmuls stream with the contraction dim on partitions and no on-device
    transposes are needed.
"""

import math
import os
import sys

import numpy as np

for _p in ("/opt/trn_rl_repo",):
    if os.path.isdir(_p) and _p not in sys.path:
        sys.path.insert(0, _p)

B = 8
S = 4096
E = 512
NH = 8
DH = 64
PAD = 4               # zero columns padded on each side of values^T
CHUNK = int(os.environ.get("KERNEL_CHUNK", "1024"))  # seq positions per step
NCHUNK = S // CHUNK
W_VT = CHUNK + 6      # vt tile width (chunk + conv halo +-2 + shift halo +-1)
W_U = CHUNK + 2       # conv output width (chunk + shift halo +-1)

HEAD_SHIFTS = [0, -1, 1, 0, 0, -1, 1, 0]
HEAD_ORDER = [0, 3, 4, 7, 1, 5, 2, 6]      # group heads by shift
TILE_SHIFT = [0, 0, -1, 1]                 # uniform shift per 128-chan tile

_c = [math.exp(-0.5 * d * d) / math.sqrt(2.0 * math.pi) for d in (0, 1, 2)]
C0, C1, C2 = _c
R21 = C2 / C1
R10 = C1 / C0
R20 = C2 / C0

# matmul streaming dtype: float32r runs at full PE rate (fp32 runs 1/4 rate)
MM_DTYPE = os.environ.get("KERNEL_MM_DTYPE", "float32r")
# bf16 conv: values/Wi/conv pipeline in bfloat16 (halves DVE time + vt DMA);
# matmul2 stays in MM_DTYPE so only the first projection sees bf16 inputs
CONV_BF16 = os.environ.get("KERNEL_CONV_BF16", "0") == "1"
# how many of the 8 conv adds per chunk run on gpsimd (walrus only allows
# plain tensor_tensor add/mult on Pool; the fused STT ops stay on DVE)
POOL_ADDS = int(os.environ.get("KERNEL_POOL_ADDS", "5"))

_CACHE = {}


def _build_bass():
    import concourse.bacc as bacc
    import concourse.mybir as mybir
    from concourse.tile import TileContext

    f32 = mybir.dt.float32
    mmdt = getattr(mybir.dt, MM_DTYPE)
    bf16 = mybir.dt.bfloat16
    cdt = bf16 if CONV_BF16 else f32      # conv pipeline dtype
    m1dt = bf16 if CONV_BF16 else mmdt    # matmul1 operand dtype
    ADD = mybir.AluOpType.add
    MUL = mybir.AluOpType.mult

    nc = bacc.Bacc(trn_type="TRN2")
    # tensors feeding the PE are declared in the matmul dtype (float32r is
    # bit-compatible with float32 host-side)
    vt = nc.dram_tensor("vt", [E, S + 2 * PAD], cdt, kind="ExternalInput")
    wi = nc.dram_tensor("wi", [E, E], m1dt, kind="ExternalInput")  # (c0*Wi_perm).T
    wo = nc.dram_tensor("wo", [E, E], mmdt, kind="ExternalInput")  # Wo[:, perm].T
    out = nc.dram_tensor("out", [S, E], f32, kind="ExternalOutput")

    with TileContext(nc) as tc:
        with (
            tc.tile_pool(name="wts", bufs=1) as wpool,
            tc.tile_pool(name="vtp", bufs=2) as vtpool,
            tc.tile_pool(name="cvp", bufs=4) as cvpool,
            tc.tile_pool(name="up", bufs=2) as upool,
            tc.tile_pool(name="y2p", bufs=2) as y2pool,
            tc.tile_pool(name="osp", bufs=8) as ospool,
            tc.tile_pool(name="ps1p", bufs=4, space="PSUM") as ps1pool,
            tc.tile_pool(name="ps2p", bufs=4, space="PSUM") as ps2pool,
        ):
            wi_sb = []
            wo_sb = []
            for j in range(4):
                wti = wpool.tile([128, E], m1dt, name=f"wi{j}", tag=f"wi{j}")
                nc.scalar.dma_start(out=wti[:, :], in_=wi[128 * j : 128 * (j + 1), :])
                wi_sb.append(wti)
                wto = wpool.tile([128, E], mmdt, name=f"wo{j}", tag=f"wo{j}")
                nc.scalar.dma_start(out=wto[:, :], in_=wo[128 * j : 128 * (j + 1), :])
                wo_sb.append(wto)

            for ci in range(NCHUNK):
                base = ci * CHUNK
                # load values^T tiles: local col L <-> padded col base+1+L,
                # i.e. unpadded seq position base + L - 3
                vts = []
                for j in range(4):
                    t = vtpool.tile([128, W_VT], cdt, name=f"vt{ci}_{j}", tag=f"vt{j}")
                    nc.sync.dma_start(
                        out=t[:, :],
                        in_=vt[128 * j : 128 * (j + 1), base + 1 : base + 1 + W_VT],
                    )
                    vts.append(t)

                # conv0 (u' = x + (c1/c0)*(x[-1]+x[+1]) + (c2/c0)*(x[-2]+x[+2]))
                # u local col U <-> seq position base - 1 + U
                us = []
                for j in range(4):
                    v = vts[j]
                    e1 = nc.gpsimd if (2 * j) < POOL_ADDS else nc.vector
                    e2 = nc.gpsimd if (2 * j + 1) < POOL_ADDS else nc.vector
                    a1 = cvpool.tile([128, W_U], cdt, name=f"a1_{ci}_{j}", tag="a1")
                    a2 = cvpool.tile([128, W_U], cdt, name=f"a2_{ci}_{j}", tag="a2")
                    u = upool.tile([128, W_U], m1dt, name=f"u{ci}_{j}", tag=f"u{j}")
                    e1.tensor_tensor(a1[:, :], v[:, 1 : 1 + W_U], v[:, 3 : 3 + W_U], ADD)
                    e2.tensor_tensor(a2[:, :], v[:, 0:W_U], v[:, 4 : 4 + W_U], ADD)
                    if CONV_BF16:
                        # SCALAR_TENSOR_TENSOR only has a 1x uop; use 4x
                        # tensor_scalar muls + 2x adds instead
                        nc.vector.tensor_scalar_mul(a1[:, :], a1[:, :], R10)
                        nc.vector.tensor_scalar_mul(a2[:, :], a2[:, :], R20)
                        nc.vector.tensor_tensor(a1[:, :], a1[:, :], a2[:, :], ADD)
                        nc.vector.tensor_tensor(u[:, :], a1[:, :], v[:, 2 : 2 + W_U], ADD)
                    else:
                        nc.vector.scalar_tensor_tensor(
                            a2[:, :], a2[:, :], R21, a1[:, :], MUL, ADD
                        )
                        nc.vector.scalar_tensor_tensor(
                            u[:, :], a2[:, :], R10, v[:, 2 : 2 + W_U], MUL, ADD
                        )
                    us.append(u)

                # matmul1: y2T[tile m] = (c0*Wi_perm)[tile m] @ u'[:, s+shift]
                y2s = []
                for m in range(4):
                    y2 = y2pool.tile([128, CHUNK], mmdt, name=f"y2_{ci}_{m}", tag=f"y2{m}")
                    off = 1 + TILE_SHIFT[m]
                    for h in range(CHUNK // 512):
                        p = ps1pool.tile([128, 512], f32, name=f"p1_{ci}_{m}_{h}", tag="ps1")
                        for j in range(4):
                            nc.tensor.matmul(
                                p[:, :],
                                wi_sb[j][:, 128 * m : 128 * (m + 1)],
                                us[j][:, off + 512 * h : off + 512 * h + 512],
                                start=(j == 0),
                                stop=(j == 3),
                            )
                        nc.scalar.copy(y2[:, 512 * h : 512 * (h + 1)], p[:, :])
                    y2s.append(y2)

                # matmul2: out[s tile] = y2T^T @ WoT
                for t_i in range(CHUNK // 128):
                    p = ps2pool.tile([128, 512], f32, name=f"p2_{ci}_{t_i}", tag="ps2")
                    for jp in range(4):
                        nc.tensor.matmul(
                            p[:, :],
                            y2s[jp][:, 128 * t_i : 128 * (t_i + 1)],
                            wo_sb[jp][:, :],
                            start=(jp == 0),
                            stop=(jp == 3),
                        )
                    ot = ospool.tile([128, E], f32, name=f"ot_{ci}_{t_i}", tag="os")
                    nc.scalar.copy(ot[:, :], p[:, :])
                    nc.scalar.dma_start(
                        out=out[base + 128 * t_i : base + 128 * (t_i + 1), :],
                        in_=ot[:, :],
                    )
    nc.finalize()
    return nc


def _get_bass():
    key = (MM_DTYPE, POOL_ADDS, CHUNK, CONV_BF16)
    if key not in _CACHE:
        _CACHE[key] = _build_bass()
    return _CACHE[key]


def _host_prep(values, input_weights, out_proj_w):
    import ml_dtypes

    cnp = ml_dtypes.bfloat16 if CONV_BF16 else np.float32
    perm = np.concatenate([np.arange(h * DH, (h + 1) * DH) for h in HEAD_ORDER])
    wi_t = np.ascontiguousarray((C0 * input_weights[perm, :]).T).astype(cnp)
    wo_t = np.ascontiguousarray(out_proj_w[:, perm].T, dtype=np.float32)
    in_maps = []
    for b in range(B):
        vtp = np.zeros((E, S + 2 * PAD), dtype=cnp)
        vtp[:, PAD : PAD + S] = values[b].T.astype(cnp)
        in_maps.append({"vt": vtp, "wi": wi_t, "wo": wo_t})
    return in_maps


def _install_profile_shim():
    """Provide the antenv.axon_hooks module bass_utils expects for NTFF
    tracing under axon, and stub out the S3 artifact upload."""
    import types

    try:
        import antenv.axon_hooks  # noqa: F401

        have = True
    except ImportError:
        have = False
    if not have:
        mod = types.ModuleType("antenv.axon_hooks")
        mod._hook = None

        def set_axon_ntff_profile_hook(h):
            mod._hook = h

        def get_axon_ntff_profile_hook():
            return mod._hook

        mod.set_axon_ntff_profile_hook = set_axon_ntff_profile_hook
        mod.get_axon_ntff_profile_hook = get_axon_ntff_profile_hook
        sys.modules["antenv.axon_hooks"] = mod
        import antenv

        antenv.axon_hooks = mod
        try:
            from trn_agent_boot.trn_boot import _ntff_profile_via_ctypes

            mod._hook = _ntff_profile_via_ctypes("/opt/axon/libaxon_pjrt.so")
        except Exception as e:  # pragma: no cover
            print(f"ntff hook install failed: {e}", file=sys.stderr)

    import concourse.bass_utils as bu

    bu.upload_artifacts = lambda tmpdir: tmpdir


def kernel(**inputs):
    values = np.asarray(inputs["values"], dtype=np.float32)
    wi = np.asarray(inputs["input_weights"], dtype=np.float32)
    wo = np.asarray(inputs["out_proj_w"], dtype=np.float32)
    assert values.shape == (B, S, E), values.shape

    trace = os.environ.get("KERNEL_TRACE", "0") == "1"
    if trace:
        try:
            _install_profile_shim()
        except Exception as e:
            print(f"profile shim failed ({e}); tracing disabled", file=sys.stderr)
            trace = False

    from concourse.bass_utils import run_bass_kernel_spmd

    nc = _get_bass()
    in_maps = _host_prep(values, wi, wo)
    res = run_bass_kernel_spmd(nc, in_maps, core_ids=list(range(B)), trace=trace)
    if trace and res.exec_time_ns is not None:
        print(f"HW exec time: {res.exec_time_ns} ns")
        kernel.last_exec_time_ns = res.exec_time_ns
    out = np.stack([res.results[b]["out"] for b in range(B)], axis=0)
    return out


kernel.last_exec_time_ns = None
